# revision 33
# baseline (speedup 1.0000x reference)
"""LITv1 transformer block on 8 TRN2 NeuronCores, data-parallel over batch.

v3 layout strategy (per core, 8 batches x 256 tokens):
- all matmuls in bf16 (weights converted host-side; activations written bf16
  by the producing engine) -> halves DMA traffic, full PE rate, rel err ~2e-3
- LN rstd via exp(-0.5*ln(var+eps)); the {Ln, Exp, Copy} table is pre-placed
  once so the auto-inserter does not thrash; phase B LN stats/rstd are
  precomputed in phase A so phase B is pure {Gelu, Copy}
- LN-output transposes via the DMA XBAR (dma_start_transpose, bf16)
- attention in head pairs; per (pair, head) PSUM bank holds both key chunks
  (all matmuls into one bank share a tile position - mixing 0/64 row offsets
  in one bank crashes the PE)
- weights (wqkv, wproj) and exp(bias) table loaded once, resident
- software pipelining: batch b+1's x-load/LN/transpose emitted during batch
  b's attention; block n+1's r1-load/LN2/transpose + first fc1 weight slice
  emitted during block n; fc2 weight chunks double-buffered
- dep-stalled DMAs hold their issuing sequencer, so compute-dependent
  stores (r1, y) are emitted after the next stage's loads are queued
  (NOTE: issuing stores on the Activation DMA queue corrupts results on
  HW - keep all DMA on nc.sync); block 0's MLP inputs are precomputed in
  phase A to hide the phase seam
"""
import sys

import numpy as np

sys.path.insert(0, "/opt/trn_rl_repo")

import ml_dtypes  # noqa: E402

import concourse.bass as bass  # noqa: E402
import concourse.mybir as mybir  # noqa: E402
import concourse.tile as tile  # noqa: E402
from concourse import bacc  # noqa: E402
from concourse.bass_utils import run_bass_kernel_spmd  # noqa: E402

F32 = mybir.dt.float32
F32R = mybir.dt.float32r
BF16 = mybir.dt.bfloat16
AF = mybir.ActivationFunctionType
ALU = mybir.AluOpType

B, N, C = 64, 256, 1024
H, DH = 16, 64
DFF = 4 * C
NCORES = 8
BLOC = B // NCORES          # 8 batches per core
TOK = BLOC * N              # 2048 tokens per core
KC = C // 128               # 8 contraction chunks
NPAIR = H // 2              # 8 head pairs


def _prep_a(nc, P, cst, x_d, b, mid_fn=None):
    """x load + LN1 + XBAR transpose for batch b -> (x_tiles, xnT)."""
    (pax, pan, pat, paq, pav, pae, pap, pad_, par, pao, pas, pa1) = P[:12]
    eps_sb = cst[2]
    t0 = b * N
    x_tiles = []
    for t in range(2):
        xt = pax.tile([128, C], F32, tag="x", name="xt")
        nc.sync.dma_start(xt, x_d[t0 + t * 128: t0 + (t + 1) * 128, :])
        x_tiles.append(xt)
    if mid_fn is not None:
        mid_fn()    # dep-free loads queue ahead of the dep-stalled transposes
    xnT = []
    for t in range(2):
        xt = x_tiles[t]
        stats = pa1.tile([128, 2, 6], F32, tag="st1", name="stats")
        xv = xt.rearrange("p (s f) -> p s f", s=2)
        for s in range(2):
            nc.vector.bn_stats(stats[:, s, :], xv[:, s, :])
        mv = pa1.tile([128, 2], F32, tag="mv1", name="mv")
        nc.vector.bn_aggr(mv, stats)
        tln = pa1.tile([128, 1], F32, tag="tln1", name="tln")
        nc.scalar.activation(tln, mv[:, 1:2], AF.Ln, bias=eps_sb, scale=1.0)
        rstd = pa1.tile([128, 1], F32, tag="rstd1", name="rstd")
        nc.scalar.activation(rstd, tln, AF.Exp, bias=0.0, scale=-0.5)
        xn = pan.tile([128, C], BF16, tag="xn", name="xn")
        nc.vector.tensor_scalar(xn, xt, mv[:, 0:1], rstd,
                                ALU.subtract, ALU.mult)
        xT = pat.tile([128, KC, 128], BF16, tag="xnT", name="xT")
        nc.sync.dma_start_transpose(xT, xn)
        xnT.append(xT)
    return x_tiles, xnT


def _body_a(nc, P, cst, prep, next_prep_fn, r1_dram, mv2_sb,
            r1_first, xnT2_first, b):
    """QKV + attention + proj + residual for batch b."""
    (pax, pan, pat, paq, pav, pae, pap, pad_, par, pao, pas, pa1,
     psW, psS, psO) = P
    (ones2, onec, eps_sb, wqkv_sb, wproj_sb, expb_sb) = cst
    x_tiles, xnT = prep
    t0 = b * N

    # ---- QKV ----
    qkT = paq.tile([128, 2 * KC, N], BF16, tag="qkT", name="qkT")
    for f in range(KC):       # fills of 2 chunks (2f, 2f+1)
        qk = psW.tile([128, 2, N], F32, tag="w512", name="qk")
        for c2 in range(2):
            co = 2 * f + c2
            for t in range(2):
                for k in range(KC):
                    nc.tensor.matmul(
                        qk[:, c2, t * 128:(t + 1) * 128],
                        wqkv_sb[:, k, co * 128:(co + 1) * 128],
                        xnT[t][:, k, :],
                        start=(k == 0), stop=(k == KC - 1),
                    )
        nc.scalar.copy(qkT[:, 2 * f:2 * f + 2, :], qk)
    # V token-major with ones column at dh index 64
    v_sb = pav.tile([128, 2, H, 66], BF16, tag="v", name="v_sb")
    for t in range(2):
        nc.vector.tensor_copy(v_sb[:, t, :, 64:65], onec.unsqueeze(2))
        for vc in range(2):
            vp = psW.tile([128, 512], F32, tag="w512", name="vp")
            for k in range(KC):
                nc.tensor.matmul(
                    vp,
                    xnT[t][:, k, :],
                    wqkv_sb[:, k, 2 * C + vc * 512: 2 * C + (vc + 1) * 512],
                    start=(k == 0), stop=(k == KC - 1),
                )
            nc.scalar.copy(
                v_sb[:, t, vc * 8:(vc + 1) * 8, 0:64],
                vp.rearrange("p (h d) -> p h d", h=8),
            )

    # prefetch next batch's LN pipeline while attention runs
    nxt = next_prep_fn() if next_prep_fn is not None else None

    # ---- attention, head pairs (2g, 2g+1), software pipeline ----
    oall = pao.tile([128, KC, N], BF16, tag="oall", name="oall")
    p_tiles, op_tiles = {}, {}

    def emit_scores(g):
        for h2 in range(2):
            base = 64 * h2
            sp = psS.tile([128, 2, N], F32, tag="sp", name="sp")
            for nk in range(2):
                nc.tensor.matmul(
                    sp[:, nk, :],
                    qkT[base:base + 64, KC + g, nk * 128:(nk + 1) * 128],
                    qkT[base:base + 64, g, :],
                    start=True, stop=True,
                )
            e = pae.tile([128, 2, N], BF16, tag="e", name="e")
            nc.scalar.activation(e, sp, AF.Exp, bias=0.0, scale=0.125)
            p = pap.tile([128, 2, N], BF16, tag="p", name="p")
            nc.vector.tensor_mul(p, e, expb_sb[:, g, h2, :, :])
            p_tiles[(g, h2)] = p

    def emit_pv(g):
        # copy PV out of PSUM immediately so the bank frees after ~1us
        # instead of waiting for the whole 3-engine norm chain. Head h2=0
        # lands at partition base 0 (denominator row 64 in col 0), head
        # h2=1 is shifted to base 64 (its denominator row goes to col 2 at
        # partition 64) so every later SBUF-SBUF op has matching bases.
        o_sb = pao.tile([128, 3, N], F32R, tag="osb", name="o_sb", bufs=3)
        for h2 in range(2):
            op = psO.tile([128, N], F32, tag="o256", name="op")
            for nk in range(2):
                nc.tensor.matmul(
                    op[0:65, :],
                    v_sb[:, nk, 2 * g + h2, 0:65],
                    p_tiles[(g, h2)][:, nk, :],
                    start=(nk == 0), stop=(nk == 1),
                )
            if h2 == 0:
                nc.scalar.copy(o_sb[0:65, 0, :], op[0:65, :])
            else:
                # DVE for head 1 so Act only carries exp + one copy per pair
                nc.vector.tensor_copy(o_sb[64:128, 1, :], op[0:64, :])
                nc.vector.tensor_copy(o_sb[64:65, 2, :], op[64:65, :])
        op_tiles[g] = o_sb

    def emit_norm(g):
        o_sb = op_tiles[g]
        bc = psW.tile([128, N], F32, tag="w512", name="bc")
        nc.tensor.matmul(bc, ones2[64:65, 0, :], o_sb[64:65, 0, :],
                         start=True, stop=False)
        nc.tensor.matmul(bc, ones2[64:65, 1, :], o_sb[64:65, 2, :],
                         start=False, stop=True)
        rd = par.tile([128, N], F32, tag="rd", name="rd")
        nc.vector.reciprocal(rd, bc)
        for h2 in range(2):
            base = 64 * h2
            # normalize on the otherwise-idle Pool engine (all-SBUF op)
            nc.gpsimd.tensor_mul(
                oall[base:base + 64, g, :],
                o_sb[base:base + 64, h2, :].bitcast(F32),
                rd[base:base + 64, :],
            )

    for g in range(NPAIR + 4):
        if g < NPAIR:
            emit_scores(g)
        if 3 <= g < NPAIR + 3:
            emit_pv(g - 3)
        if g >= 4:
            emit_norm(g - 4)

    # ---- proj + residual -> r1 + LN2 stats (for phase B) ----
    for t in range(2):
        st = pas.tile([128, C], F32, tag="stg", name="st")
        for co in range(2):
            pp = psW.tile([128, 512], F32, tag="w512", name="pp")
            for k in range(KC):
                nc.tensor.matmul(
                    pp,
                    oall[:, k, t * 128:(t + 1) * 128],
                    wproj_sb[:, k, co * 512:(co + 1) * 512],
                    start=(k == 0), stop=(k == KC - 1),
                )
            nc.vector.tensor_add(
                st[:, co * 512:(co + 1) * 512],
                pp,
                x_tiles[t][:, co * 512:(co + 1) * 512],
            )
        idx = b * 2 + t
        stats2 = pa1.tile([128, 2, 6], F32, tag="st2", name="stats2")
        sv = st.rearrange("p (s f) -> p s f", s=2)
        for s in range(2):
            nc.vector.bn_stats(stats2[:, s, :], sv[:, s, :])
        mv2 = pa1.tile([128, 2], F32, tag="mv2", name="mv2")
        nc.vector.bn_aggr(mv2, stats2)
        nc.vector.tensor_copy(mv2_sb[:, idx, 0:1], mv2[:, 0:1])
        tln2 = pa1.tile([128, 1], F32, tag="tln2", name="tln2")
        nc.scalar.activation(tln2, mv2[:, 1:2], AF.Ln, bias=eps_sb, scale=1.0)
        nc.scalar.activation(mv2_sb[:, idx, 1:2], tln2, AF.Exp,
                             bias=0.0, scale=-0.5)
        if b < 2:
            # block 0 of phase B: keep residual + normalized-transposed
            # input resident so the MLP can start right at the seam
            nc.vector.tensor_copy(r1_first[:, idx, :], st)
            xn2f = pan.tile([128, C], BF16, tag="xn", name="xn2f")
            nc.vector.tensor_scalar(
                xn2f, st, mv2[:, 0:1], mv2_sb[:, idx, 1:2],
                ALU.subtract, ALU.mult,
            )
            nc.sync.dma_start_transpose(xnT2_first[:, idx, :, :], xn2f)
        # compute-dependent store: Activation DMA queue so it cannot
        # dep-stall the SP load queue
        nc.sync.dma_start(
            r1_dram[t0 + t * 128: t0 + (t + 1) * 128, :], st
        )
    return nxt


def _preload_wf1(nc, wf1_pre, wfc1_d):
    """Preload block 0's first fc1 weight slices during phase A."""
    for i, wt in enumerate(wf1_pre):
        nc.sync.dma_start(
            wt,
            wfc1_d[:, i * 512:(i + 1) * 512].rearrange(
                "(k p) n -> p k n", p=128
            ),
        )


def _phase_b(nc, P, wfc1_d, wfc2_d, r1_dram, mv2_sb, y_d,
             r1_first, xnT2_first, wf1_pre, psW, psS, psO):
    """MLP + residual for all 4 token blocks, software-pipelined."""
    (pbr, pbn, pbt, pbw1, pbw2, pby) = P
    NB = 4
    BT = TOK // NB
    KF = DFF // 128

    def wf1_dma(s):
        wf1 = pbw1.tile([128, KC, 512], BF16, tag="wf1", name="wf1")
        nc.sync.dma_start(
            wf1,
            wfc1_d[:, s * 512:(s + 1) * 512].rearrange(
                "(k p) n -> p k n", p=128
            ),
        )
        return wf1

    def prep(blk):
        """r1 loads + LN2 + transposes + first fc1 weight slices for blk."""
        t0 = blk * BT
        r1_tiles, xnT2 = [], []
        for t in range(4):
            idx = blk * 4 + t
            rt = pbr.tile([128, C], F32, tag="r1", name="rt")
            nc.sync.dma_start(
                rt, r1_dram[t0 + t * 128: t0 + (t + 1) * 128, :]
            )
            xn2 = pbn.tile([128, C], BF16, tag="xn2", name="xn2")
            nc.vector.tensor_scalar(
                xn2, rt, mv2_sb[:, idx, 0:1], mv2_sb[:, idx, 1:2],
                ALU.subtract, ALU.mult,
            )
            xT2 = pbt.tile([128, KC, 128], BF16, tag="xnT2", name="xT2")
            nc.sync.dma_start_transpose(xT2, xn2)
            r1_tiles.append(rt)
            xnT2.append(xT2)
        return r1_tiles, xnT2, [wf1_dma(0), wf1_dma(1)]

    # block 0 inputs were produced during phase A (incl. wf1 slices 0/1)
    cur = ([r1_first[:, t, :] for t in range(4)],
           [xnT2_first[:, t, :, :] for t in range(4)],
           list(wf1_pre))
    pending_stores = []

    for blk in range(NB):
        t0 = blk * BT
        r1_tiles, xnT2, wf1s = cur

        # previous block's y stores go out now, after this block's first
        # weight slices are queued (they dep-stall the SP sequencer)
        for args in pending_stores:
            nc.sync.dma_start(*args)
        pending_stores = []

        # fc1 + gelu -> hT, weights prefetched 2 slices ahead
        hT = pbt.tile([128, DFF // 128, BT], BF16, tag="hT", name="hT",
                      bufs=1)
        for s in range(8):
            if s + 2 < 8:
                wf1s.append(wf1_dma(s + 2))
            wf1 = wf1s[s]
            for dc in range(4):
                fp = psW.tile([128, BT], F32, tag="w512", name="fp")
                for t in range(4):
                    for k in range(KC):
                        nc.tensor.matmul(
                            fp[:, t * 128:(t + 1) * 128],
                            wf1[:, k, dc * 128:(dc + 1) * 128],
                            xnT2[t][:, k, :],
                            start=(k == 0), stop=(k == KC - 1),
                        )
                nc.scalar.activation(
                    hT[:, s * 4 + dc, :], fp, AF.Gelu_apprx_tanh
                )

        # prefetch next block's inputs before fc2's weight streaming
        if blk + 1 < NB:
            cur = prep(blk + 1)

        # fc2 + residual -> y (wfc2 chunks double-buffered)
        wf2 = pbw2.tile([128, KF // 4, 512], BF16, tag="wf2", name="wf2")
        nc.sync.dma_start(
            wf2, wfc2_d[0:DFF // 4, 0:512].rearrange("(k p) n -> p k n", p=128)
        )
        for co in range(2):
            op2s = [
                (psS if t < 2 else psO).tile(
                    [128, 512], F32, tag=("sp" if t < 2 else "o256"),
                    name=f"op2_{t}")
                for t in range(4)
            ]
            for kh in range(4):
                if kh + 1 < 4 or co == 0:
                    nco, nkh = (co, kh + 1) if kh + 1 < 4 else (1, 0)
                    wf2n = pbw2.tile([128, KF // 4, 512], BF16, tag="wf2",
                                     name="wf2n")
                    nc.sync.dma_start(
                        wf2n,
                        wfc2_d[
                            nkh * (DFF // 4):(nkh + 1) * (DFF // 4),
                            nco * 512:(nco + 1) * 512,
                        ].rearrange("(k p) n -> p k n", p=128),
                    )
                for t in range(4):
                    for kk in range(KF // 4):
                        k = kh * (KF // 4) + kk
                        nc.tensor.matmul(
                            op2s[t],
                            hT[:, k, t * 128:(t + 1) * 128],
                            wf2[:, kk, :],
                            start=(k == 0), stop=(k == KF - 1),
                        )
                if kh + 1 < 4 or co == 0:
                    wf2 = wf2n
            for t in range(4):
                yst = pby.tile([128, 512], F32, tag="yst", name="yst")
                nc.vector.tensor_add(
                    yst, op2s[t], r1_tiles[t][:, co * 512:(co + 1) * 512]
                )
                pending_stores.append((
                    y_d[t0 + t * 128: t0 + (t + 1) * 128,
                        co * 512:(co + 1) * 512],
                    yst,
                ))
    for args in pending_stores:
        nc.sync.dma_start(*args)


_NC_CACHE = {}


def build():
    nc = bacc.Bacc("TRN2")
    x_d = nc.dram_tensor("x", [TOK, C], F32, kind="ExternalInput")
    wqkv_d = nc.dram_tensor("wqkv", [C, 3 * C], BF16, kind="ExternalInput")
    wproj_d = nc.dram_tensor("wproj", [C, C], BF16, kind="ExternalInput")
    wfc1_d = nc.dram_tensor("wfc1", [C, DFF], BF16, kind="ExternalInput")
    wfc2_d = nc.dram_tensor("wfc2", [DFF, C], BF16, kind="ExternalInput")
    # expb[p, g, h2, nk, q] = exp(bias[2g+h2, q, nk*128+p])
    expb_d = nc.dram_tensor("expb", [128, NPAIR * 2 * 2 * N], BF16,
                            kind="ExternalInput")
    y_d = nc.dram_tensor("y", [TOK, C], F32, kind="ExternalOutput")

    with tile.TileContext(nc) as tc:
        with (
            tc.tile_pool(name="consts", bufs=1) as consts,
            tc.tile_pool(name="resid", bufs=1) as resid,
            tc.tile_pool(name="dram", bufs=1, space="DRAM") as dpool,
            tc.tile_pool(name="psW", bufs=2, space="PSUM") as psW,
            tc.tile_pool(name="psS", bufs=3, space="PSUM") as psS,
            tc.tile_pool(name="psO", bufs=3, space="PSUM") as psO,
        ):
            # ones2[:, 0, 0:64] = 1, ones2[:, 1, 64:128] = 1 (denom bcast
            # row patterns, replicated on every partition so any partition
            # base can serve as the matmul contraction row)
            ones2_f = consts.tile([128, 2, 128], F32)
            nc.vector.memset(ones2_f, 0.0)
            nc.vector.memset(ones2_f[:, 0, 0:64], 1.0)
            nc.vector.memset(ones2_f[:, 1, 64:128], 1.0)
            ones2 = consts.tile([128, 2, 128], F32R)
            nc.vector.tensor_copy(ones2, ones2_f)
            onec_f = consts.tile([128, H], F32)
            nc.vector.memset(onec_f, 1.0)
            onec = consts.tile([128, H], BF16)
            nc.vector.tensor_copy(onec, onec_f)
            eps_sb = consts.tile([128, 1], F32)
            nc.vector.memset(eps_sb, 1e-5)
            # pre-place the {Ln, Exp, Copy} table (set 6) so the auto
            # inserter does not thrash between ln-only and exp-only tables
            nc.scalar.add_instruction(mybir.InstLoadActFuncSet(
                name=nc.get_next_instruction_name(), act_func_set_id=6,
                ins=[], outs=[]))

            mv2_sb = resid.tile([128, TOK // 128, 2], F32)
            r1_first = resid.tile([128, 4, C], F32)
            xnT2_first = resid.tile([128, 4, KC, 128], BF16)
            wf1_pre = [resid.tile([128, KC, 512], BF16, name=f"wf1p{i}")
                       for i in range(2)]
            r1_dram = dpool.tile([TOK, C], F32)
            _NC_CACHE["r1_name"] = r1_dram.name

            with (
                tc.tile_pool(name="paw", bufs=1) as paw,
                tc.tile_pool(name="pax", bufs=4) as pax,
                tc.tile_pool(name="pan", bufs=2) as pan,
                tc.tile_pool(name="pat", bufs=4) as pat,
                tc.tile_pool(name="paq", bufs=2) as paq,
                tc.tile_pool(name="pav", bufs=2) as pav,
                tc.tile_pool(name="pae", bufs=2) as pae,
                tc.tile_pool(name="pap", bufs=3) as pap,
                tc.tile_pool(name="pad", bufs=2) as pad_,
                tc.tile_pool(name="par", bufs=2) as par,
                tc.tile_pool(name="pao", bufs=2) as pao,
                tc.tile_pool(name="pas", bufs=2) as pas,
                tc.tile_pool(name="pa1", bufs=2) as pa1,
            ):
                wqkv_sb = paw.tile([128, KC, 3 * C], BF16)
                wproj_sb = paw.tile([128, KC, C], BF16)
                expb_sb_flat = paw.tile([128, NPAIR * 2 * 2 * N], BF16)
                expb_sb = expb_sb_flat.rearrange(
                    "p (g h nk q) -> p g h nk q", g=NPAIR, nk=2, h=2
                )
                cst = (ones2, onec, eps_sb, wqkv_sb, wproj_sb, expb_sb)
                P = (pax, pan, pat, paq, pav, pae, pap, pad_, par, pao,
                     pas, pa1, psW, psS, psO)
                # startup: x(b0), then Q+K weight sections (needed first by
                # batch 0), then LN/transposes, then the remaining residents
                def _mid():
                    # two halves: the first QK fills only need cols 0:512
                    for h in range(2):
                        nc.sync.dma_start(
                            wqkv_sb[:, :, h * 512:(h + 1) * 512],
                            wqkv_d[:, h * 512:(h + 1) * 512].rearrange(
                                "(k p) n -> p k n", p=128
                            ),
                        )
                prep = _prep_a(nc, P, cst, x_d, 0, mid_fn=_mid)
                for sec in (1, 2):
                    nc.sync.dma_start(
                        wqkv_sb[:, :, sec * C:(sec + 1) * C],
                        wqkv_d[:, sec * C:(sec + 1) * C].rearrange(
                            "(k p) n -> p k n", p=128
                        ),
                    )
                nc.sync.dma_start(expb_sb_flat, expb_d[:])
                nc.sync.dma_start(
                    wproj_sb, wproj_d[:].rearrange("(k p) n -> p k n", p=128)
                )
                for b in range(BLOC):
                    nxt_fn = (
                        (lambda bb=b: _prep_a(nc, P, cst, x_d, bb + 1))
                        if b + 1 < BLOC else None
                    )
                    prep = _body_a(nc, P, cst, prep, nxt_fn, r1_dram,
                                   mv2_sb, r1_first, xnT2_first, b)
                    if b == 5:
                        _preload_wf1(nc, wf1_pre, wfc1_d)

            with (
                tc.tile_pool(name="pbr", bufs=9) as pbr,
                tc.tile_pool(name="pbn", bufs=3) as pbn,
                tc.tile_pool(name="pbt", bufs=9) as pbt,
                tc.tile_pool(name="pbw1", bufs=3) as pbw1,
                tc.tile_pool(name="pbw2", bufs=3) as pbw2,
                tc.tile_pool(name="pby", bufs=2) as pby,
            ):
                P = (pbr, pbn, pbt, pbw1, pbw2, pby)
                _phase_b(nc, P, wfc1_d, wfc2_d, r1_dram, mv2_sb, y_d,
                         r1_first, xnT2_first, wf1_pre, psW, psS, psO)

    nc.finalize()
    return nc


def _get_nc():
    if "nc" not in _NC_CACHE:
        _NC_CACHE["nc"] = build()
    return _NC_CACHE["nc"]


def kernel(**inputs):
    x = np.asarray(inputs["x"], dtype=np.float32)
    qkv_w = np.asarray(inputs["qkv_w"], dtype=np.float32)
    qkv_b = np.asarray(inputs["qkv_b"], dtype=np.float32)
    proj_w = np.asarray(inputs["proj_w"], dtype=np.float32)
    proj_b = np.asarray(inputs["proj_b"], dtype=np.float32)
    fc1_w = np.asarray(inputs["fc1_w"], dtype=np.float32)
    fc1_b = np.asarray(inputs["fc1_b"], dtype=np.float32)
    fc2_w = np.asarray(inputs["fc2_w"], dtype=np.float32)
    fc2_b = np.asarray(inputs["fc2_b"], dtype=np.float32)
    ln1_g = np.asarray(inputs["ln1_g"], dtype=np.float32)
    ln1_b = np.asarray(inputs["ln1_b"], dtype=np.float32)
    ln2_g = np.asarray(inputs["ln2_g"], dtype=np.float32)
    ln2_b = np.asarray(inputs["ln2_b"], dtype=np.float32)
    rel_pos_bias = np.asarray(inputs["rel_pos_bias"], dtype=np.float32)
    rel_pos_idx = np.asarray(inputs["rel_pos_idx"])

    assert not np.any(qkv_b) and not np.any(proj_b), "nonzero bias unsupported"
    assert not np.any(fc1_b) and not np.any(fc2_b), "nonzero bias unsupported"
    assert not np.any(ln1_b) and not np.any(ln2_b), "nonzero LN bias unsupported"

    bf = ml_dtypes.bfloat16
    # fold LN gammas into the following weight matrices (exact when g == 1)
    wqkv = np.ascontiguousarray(ln1_g[:, None] * qkv_w).astype(bf)
    wproj = np.ascontiguousarray(proj_w).astype(bf)
    wfc1 = np.ascontiguousarray(ln2_g[:, None] * fc1_w).astype(bf)
    wfc2 = np.ascontiguousarray(fc2_w).astype(bf)

    # expb[p, g, h2, nk, q] = exp(bias[q, nk*128+p, 2g+h2])
    Bm = rel_pos_bias[rel_pos_idx].reshape(N, N, H)          # [q, k, h]
    E = np.exp(Bm).transpose(1, 2, 0)                        # [k, h, q]
    E = E.reshape(2, 128, NPAIR, 2, N).transpose(1, 2, 3, 0, 4)
    expb = np.ascontiguousarray(E.reshape(128, NPAIR * 2 * 2 * N)).astype(bf)

    nc = _get_nc()
    in_maps = []
    for c in range(NCORES):
        xs = np.ascontiguousarray(
            x[c * BLOC:(c + 1) * BLOC].reshape(TOK, C)
        ).astype(np.float32)
        in_maps.append(
            dict(x=xs, wqkv=wqkv, wproj=wproj, wfc1=wfc1, wfc2=wfc2,
                 expb=expb)
        )
    res = run_bass_kernel_spmd(nc, in_maps, core_ids=list(range(NCORES)))
    y = np.concatenate([res.results[c]["y"] for c in range(NCORES)], axis=0)
    return y.reshape(B, N, C).astype(np.float32)


# revision 34
# speedup vs baseline: 1.0009x; 1.0009x over previous
"""LITv1 transformer block on 8 TRN2 NeuronCores, data-parallel over batch.

v3 layout strategy (per core, 8 batches x 256 tokens):
- all matmuls in bf16 (weights converted host-side; activations written bf16
  by the producing engine) -> halves DMA traffic, full PE rate, rel err ~2e-3
- LN rstd via exp(-0.5*ln(var+eps)); the {Ln, Exp, Copy} table is pre-placed
  once so the auto-inserter does not thrash; phase B LN stats/rstd are
  precomputed in phase A so phase B is pure {Gelu, Copy}
- LN-output transposes via the DMA XBAR (dma_start_transpose, bf16)
- attention in head pairs; per (pair, head) PSUM bank holds both key chunks
  (all matmuls into one bank share a tile position - mixing 0/64 row offsets
  in one bank crashes the PE)
- weights (wqkv, wproj) and exp(bias) table loaded once, resident
- software pipelining: batch b+1's x-load/LN/transpose emitted during batch
  b's attention; block n+1's r1-load/LN2/transpose + first fc1 weight slice
  emitted during block n; fc2 weight chunks double-buffered
- dep-stalled DMAs hold their issuing sequencer, so compute-dependent
  stores (r1, y) are emitted after the next stage's loads are queued
  (NOTE: issuing stores on the Activation DMA queue corrupts results on
  HW - keep all DMA on nc.sync); block 0's MLP inputs are precomputed in
  phase A to hide the phase seam
"""
import sys

import numpy as np

sys.path.insert(0, "/opt/trn_rl_repo")

import ml_dtypes  # noqa: E402

import concourse.bass as bass  # noqa: E402
import concourse.mybir as mybir  # noqa: E402
import concourse.tile as tile  # noqa: E402
from concourse import bacc  # noqa: E402
from concourse.bass_utils import run_bass_kernel_spmd  # noqa: E402

F32 = mybir.dt.float32
F32R = mybir.dt.float32r
BF16 = mybir.dt.bfloat16
AF = mybir.ActivationFunctionType
ALU = mybir.AluOpType

B, N, C = 64, 256, 1024
H, DH = 16, 64
DFF = 4 * C
NCORES = 8
BLOC = B // NCORES          # 8 batches per core
TOK = BLOC * N              # 2048 tokens per core
KC = C // 128               # 8 contraction chunks
NPAIR = H // 2              # 8 head pairs


def _prep_a(nc, P, cst, x_d, b, mid_fn=None):
    """x load + LN1 + XBAR transpose for batch b -> (x_tiles, xnT)."""
    (pax, pan, pat, paq, pav, pae, pap, pad_, par, pao, pas, pa1) = P[:12]
    eps_sb = cst[2]
    t0 = b * N
    x_tiles = []
    for t in range(2):
        xt = pax.tile([128, C], F32, tag="x", name="xt")
        nc.sync.dma_start(xt, x_d[t0 + t * 128: t0 + (t + 1) * 128, :])
        x_tiles.append(xt)
    if mid_fn is not None:
        mid_fn()    # dep-free loads queue ahead of the dep-stalled transposes
    xnT = []
    for t in range(2):
        xt = x_tiles[t]
        stats = pa1.tile([128, 2, 6], F32, tag="st1", name="stats")
        xv = xt.rearrange("p (s f) -> p s f", s=2)
        for s in range(2):
            nc.vector.bn_stats(stats[:, s, :], xv[:, s, :])
        mv = pa1.tile([128, 2], F32, tag="mv1", name="mv")
        nc.vector.bn_aggr(mv, stats)
        tln = pa1.tile([128, 1], F32, tag="tln1", name="tln")
        nc.scalar.activation(tln, mv[:, 1:2], AF.Ln, bias=eps_sb, scale=1.0)
        rstd = pa1.tile([128, 1], F32, tag="rstd1", name="rstd")
        nc.scalar.activation(rstd, tln, AF.Exp, bias=0.0, scale=-0.5)
        xn = pan.tile([128, C], BF16, tag="xn", name="xn")
        nc.vector.tensor_scalar(xn, xt, mv[:, 0:1], rstd,
                                ALU.subtract, ALU.mult)
        xT = pat.tile([128, KC, 128], BF16, tag="xnT", name="xT")
        nc.sync.dma_start_transpose(xT, xn)
        xnT.append(xT)
    return x_tiles, xnT


def _body_a(nc, P, cst, prep, next_prep_fn, r1_dram, mv2_sb,
            r1_first, xnT2_first, b):
    """QKV + attention + proj + residual for batch b."""
    (pax, pan, pat, paq, pav, pae, pap, pad_, par, pao, pas, pa1,
     psW, psS, psO) = P
    (ones2, onec, eps_sb, wqkv_sb, wproj_sb, expb_sb) = cst
    x_tiles, xnT = prep
    t0 = b * N

    # ---- QKV ----
    qkT = paq.tile([128, 2 * KC, N], BF16, tag="qkT", name="qkT")
    for f in range(KC):       # fills of 2 chunks (2f, 2f+1)
        qk = psW.tile([128, 2, N], F32, tag="w512", name="qk")
        for c2 in range(2):
            co = 2 * f + c2
            for t in range(2):
                for k in range(KC):
                    nc.tensor.matmul(
                        qk[:, c2, t * 128:(t + 1) * 128],
                        wqkv_sb[:, k, co * 128:(co + 1) * 128],
                        xnT[t][:, k, :],
                        start=(k == 0), stop=(k == KC - 1),
                    )
        nc.scalar.copy(qkT[:, 2 * f:2 * f + 2, :], qk)
    # V token-major with ones column at dh index 64
    v_sb = pav.tile([128, 2, H, 66], BF16, tag="v", name="v_sb")
    for t in range(2):
        nc.vector.tensor_copy(v_sb[:, t, :, 64:65], onec.unsqueeze(2))
        for vc in range(2):
            vp = psW.tile([128, 512], F32, tag="w512", name="vp")
            for k in range(KC):
                nc.tensor.matmul(
                    vp,
                    xnT[t][:, k, :],
                    wqkv_sb[:, k, 2 * C + vc * 512: 2 * C + (vc + 1) * 512],
                    start=(k == 0), stop=(k == KC - 1),
                )
            nc.scalar.copy(
                v_sb[:, t, vc * 8:(vc + 1) * 8, 0:64],
                vp.rearrange("p (h d) -> p h d", h=8),
            )

    # prefetch next batch's LN pipeline while attention runs
    nxt = next_prep_fn() if next_prep_fn is not None else None

    # ---- attention, head pairs (2g, 2g+1), software pipeline ----
    oall = pao.tile([128, KC, N], BF16, tag="oall", name="oall")
    p_tiles, op_tiles = {}, {}

    def emit_scores(g):
        for h2 in range(2):
            base = 64 * h2
            sp = psS.tile([128, 2, N], F32, tag="sp", name="sp")
            for nk in range(2):
                nc.tensor.matmul(
                    sp[:, nk, :],
                    qkT[base:base + 64, KC + g, nk * 128:(nk + 1) * 128],
                    qkT[base:base + 64, g, :],
                    start=True, stop=True,
                )
            e = pae.tile([128, 2, N], BF16, tag="e", name="e")
            nc.scalar.activation(e, sp, AF.Exp, bias=0.0, scale=0.125)
            p = pap.tile([128, 2, N], BF16, tag="p", name="p")
            nc.vector.tensor_mul(p, e, expb_sb[:, g, h2, :, :])
            p_tiles[(g, h2)] = p

    def emit_pv(g):
        # copy PV out of PSUM immediately so the bank frees after ~1us
        # instead of waiting for the whole 3-engine norm chain. Head h2=0
        # lands at partition base 0 (denominator row 64 in col 0), head
        # h2=1 is shifted to base 64 (its denominator row goes to col 2 at
        # partition 64) so every later SBUF-SBUF op has matching bases.
        o_sb = pao.tile([128, 3, N], F32R, tag="osb", name="o_sb", bufs=3)
        for h2 in range(2):
            op = psO.tile([128, N], F32, tag="o256", name="op")
            for nk in range(2):
                nc.tensor.matmul(
                    op[0:65, :],
                    v_sb[:, nk, 2 * g + h2, 0:65],
                    p_tiles[(g, h2)][:, nk, :],
                    start=(nk == 0), stop=(nk == 1),
                )
            if h2 == 0:
                nc.scalar.copy(o_sb[0:65, 0, :], op[0:65, :])
            else:
                # DVE for head 1 so Act only carries exp + one copy per pair
                nc.vector.tensor_copy(o_sb[64:128, 1, :], op[0:64, :])
                nc.vector.tensor_copy(o_sb[64:65, 2, :], op[64:65, :])
        op_tiles[g] = o_sb

    def emit_norm(g):
        o_sb = op_tiles[g]
        bc = psW.tile([128, N], F32, tag="w512", name="bc")
        nc.tensor.matmul(bc, ones2[64:65, 0, :], o_sb[64:65, 0, :],
                         start=True, stop=False)
        nc.tensor.matmul(bc, ones2[64:65, 1, :], o_sb[64:65, 2, :],
                         start=False, stop=True)
        rd = par.tile([128, N], F32, tag="rd", name="rd")
        nc.vector.reciprocal(rd, bc)
        for h2 in range(2):
            base = 64 * h2
            # normalize on the otherwise-idle Pool engine (all-SBUF op)
            nc.gpsimd.tensor_mul(
                oall[base:base + 64, g, :],
                o_sb[base:base + 64, h2, :].bitcast(F32),
                rd[base:base + 64, :],
            )

    for g in range(NPAIR + 4):
        if g < NPAIR:
            emit_scores(g)
        if 3 <= g < NPAIR + 3:
            emit_pv(g - 3)
        if g >= 4:
            emit_norm(g - 4)

    # ---- proj + residual -> r1 + LN2 stats (for phase B) ----
    for t in range(2):
        st = pas.tile([128, C], F32, tag="stg", name="st")
        for co in range(2):
            pp = psW.tile([128, 512], F32, tag="w512", name="pp")
            for k in range(KC):
                nc.tensor.matmul(
                    pp,
                    oall[:, k, t * 128:(t + 1) * 128],
                    wproj_sb[:, k, co * 512:(co + 1) * 512],
                    start=(k == 0), stop=(k == KC - 1),
                )
            nc.vector.tensor_add(
                st[:, co * 512:(co + 1) * 512],
                pp,
                x_tiles[t][:, co * 512:(co + 1) * 512],
            )
        idx = b * 2 + t
        stats2 = pa1.tile([128, 2, 6], F32, tag="st2", name="stats2")
        sv = st.rearrange("p (s f) -> p s f", s=2)
        for s in range(2):
            nc.vector.bn_stats(stats2[:, s, :], sv[:, s, :])
        mv2 = pa1.tile([128, 2], F32, tag="mv2", name="mv2")
        nc.vector.bn_aggr(mv2, stats2)
        nc.vector.tensor_copy(mv2_sb[:, idx, 0:1], mv2[:, 0:1])
        tln2 = pa1.tile([128, 1], F32, tag="tln2", name="tln2")
        nc.scalar.activation(tln2, mv2[:, 1:2], AF.Ln, bias=eps_sb, scale=1.0)
        nc.scalar.activation(mv2_sb[:, idx, 1:2], tln2, AF.Exp,
                             bias=0.0, scale=-0.5)
        if b < 2:
            # block 0 of phase B: keep residual + normalized-transposed
            # input resident so the MLP can start right at the seam
            nc.vector.tensor_copy(r1_first[:, idx, :], st)
            xn2f = pan.tile([128, C], BF16, tag="xn", name="xn2f")
            nc.vector.tensor_scalar(
                xn2f, st, mv2[:, 0:1], mv2_sb[:, idx, 1:2],
                ALU.subtract, ALU.mult,
            )
            nc.sync.dma_start_transpose(xnT2_first[:, idx, :, :], xn2f)
        # compute-dependent store: Activation DMA queue so it cannot
        # dep-stall the SP load queue
        nc.sync.dma_start(
            r1_dram[t0 + t * 128: t0 + (t + 1) * 128, :], st
        )
    return nxt


def _preload_wf1(nc, wf1_pre, wfc1_d):
    """Preload block 0's first fc1 weight slices during phase A."""
    for i, wt in enumerate(wf1_pre):
        nc.sync.dma_start(
            wt,
            wfc1_d[:, i * 512:(i + 1) * 512].rearrange(
                "(k p) n -> p k n", p=128
            ),
        )


def _phase_b(nc, P, wfc1_d, wfc2_d, r1_dram, mv2_sb, y_d,
             r1_first, xnT2_first, wf1_pre, psW, psS, psO):
    """MLP + residual for all 4 token blocks, software-pipelined."""
    (pbr, pbn, pbt, pbw1, pbw2, pby) = P
    NB = 4
    BT = TOK // NB
    KF = DFF // 128

    def wf1_dma(s):
        wf1 = pbw1.tile([128, KC, 512], BF16, tag="wf1", name="wf1")
        nc.sync.dma_start(
            wf1,
            wfc1_d[:, s * 512:(s + 1) * 512].rearrange(
                "(k p) n -> p k n", p=128
            ),
        )
        return wf1

    def prep(blk):
        """r1 loads + LN2 + transposes + first fc1 weight slices for blk."""
        t0 = blk * BT
        r1_tiles, xnT2 = [], []
        for t in range(4):
            idx = blk * 4 + t
            rt = pbr.tile([128, C], F32, tag="r1", name="rt")
            nc.sync.dma_start(
                rt, r1_dram[t0 + t * 128: t0 + (t + 1) * 128, :]
            )
            xn2 = pbn.tile([128, C], BF16, tag="xn2", name="xn2")
            nc.vector.tensor_scalar(
                xn2, rt, mv2_sb[:, idx, 0:1], mv2_sb[:, idx, 1:2],
                ALU.subtract, ALU.mult,
            )
            xT2 = pbt.tile([128, KC, 128], BF16, tag="xnT2", name="xT2")
            nc.sync.dma_start_transpose(xT2, xn2)
            r1_tiles.append(rt)
            xnT2.append(xT2)
        return r1_tiles, xnT2, [wf1_dma(0), wf1_dma(1)]

    # block 0 inputs were produced during phase A (incl. wf1 slices 0/1)
    cur = ([r1_first[:, t, :] for t in range(4)],
           [xnT2_first[:, t, :, :] for t in range(4)],
           list(wf1_pre))
    pending_stores = []

    for blk in range(NB):
        t0 = blk * BT
        r1_tiles, xnT2, wf1s = cur

        # previous block's y stores go out now, after this block's first
        # weight slices are queued (they dep-stall the SP sequencer)
        for args in pending_stores:
            nc.sync.dma_start(*args)
        pending_stores = []

        # fc1 + gelu -> hT, weights prefetched 2 slices ahead
        hT = pbt.tile([128, DFF // 128, BT], BF16, tag="hT", name="hT",
                      bufs=1)
        for s in range(8):
            if s + 2 < 8:
                wf1s.append(wf1_dma(s + 2))
            wf1 = wf1s[s]
            for dc in range(4):
                fp = psW.tile([128, BT], F32, tag="w512", name="fp")
                for t in range(4):
                    for k in range(KC):
                        nc.tensor.matmul(
                            fp[:, t * 128:(t + 1) * 128],
                            wf1[:, k, dc * 128:(dc + 1) * 128],
                            xnT2[t][:, k, :],
                            start=(k == 0), stop=(k == KC - 1),
                        )
                nc.scalar.activation(
                    hT[:, s * 4 + dc, :], fp, AF.Gelu_apprx_tanh
                )

        # prefetch next block's inputs before fc2's weight streaming
        if blk + 1 < NB:
            cur = prep(blk + 1)

        # fc2 + residual -> y (wfc2 chunks double-buffered)
        wf2 = pbw2.tile([128, KF // 4, 512], BF16, tag="wf2", name="wf2")
        nc.sync.dma_start(
            wf2, wfc2_d[0:DFF // 4, 0:512].rearrange("(k p) n -> p k n", p=128)
        )
        for co in range(2):
            op2s = [
                (psS if t < 2 else psO).tile(
                    [128, 512], F32, tag=("sp" if t < 2 else "o256"),
                    name=f"op2_{t}")
                for t in range(4)
            ]
            for kh in range(4):
                if kh + 1 < 4 or co == 0:
                    nco, nkh = (co, kh + 1) if kh + 1 < 4 else (1, 0)
                    wf2n = pbw2.tile([128, KF // 4, 512], BF16, tag="wf2",
                                     name="wf2n")
                    nc.sync.dma_start(
                        wf2n,
                        wfc2_d[
                            nkh * (DFF // 4):(nkh + 1) * (DFF // 4),
                            nco * 512:(nco + 1) * 512,
                        ].rearrange("(k p) n -> p k n", p=128),
                    )
                for t in range(4):
                    for kk in range(KF // 4):
                        k = kh * (KF // 4) + kk
                        nc.tensor.matmul(
                            op2s[t],
                            hT[:, k, t * 128:(t + 1) * 128],
                            wf2[:, kk, :],
                            start=(k == 0), stop=(k == KF - 1),
                        )
                if kh + 1 < 4 or co == 0:
                    wf2 = wf2n
            for t in range(4):
                yst = pby.tile([128, 512], F32, tag="yst", name="yst")
                nc.vector.tensor_add(
                    yst, op2s[t], r1_tiles[t][:, co * 512:(co + 1) * 512]
                )
                pending_stores.append((
                    y_d[t0 + t * 128: t0 + (t + 1) * 128,
                        co * 512:(co + 1) * 512],
                    yst,
                ))
    for args in pending_stores:
        nc.sync.dma_start(*args)


_NC_CACHE = {}


def build():
    nc = bacc.Bacc("TRN2")
    x_d = nc.dram_tensor("x", [TOK, C], F32, kind="ExternalInput")
    wqkv_d = nc.dram_tensor("wqkv", [C, 3 * C], BF16, kind="ExternalInput")
    wproj_d = nc.dram_tensor("wproj", [C, C], BF16, kind="ExternalInput")
    wfc1_d = nc.dram_tensor("wfc1", [C, DFF], BF16, kind="ExternalInput")
    wfc2_d = nc.dram_tensor("wfc2", [DFF, C], BF16, kind="ExternalInput")
    # expb[p, g, h2, nk, q] = exp(bias[2g+h2, q, nk*128+p])
    expb_d = nc.dram_tensor("expb", [128, NPAIR * 2 * 2 * N], BF16,
                            kind="ExternalInput")
    y_d = nc.dram_tensor("y", [TOK, C], F32, kind="ExternalOutput")

    with tile.TileContext(nc) as tc:
        with (
            tc.tile_pool(name="consts", bufs=1) as consts,
            tc.tile_pool(name="resid", bufs=1) as resid,
            tc.tile_pool(name="dram", bufs=1, space="DRAM") as dpool,
            tc.tile_pool(name="psW", bufs=2, space="PSUM") as psW,
            tc.tile_pool(name="psS", bufs=3, space="PSUM") as psS,
            tc.tile_pool(name="psO", bufs=3, space="PSUM") as psO,
        ):
            # ones2[:, 0, 0:64] = 1, ones2[:, 1, 64:128] = 1 (denom bcast
            # row patterns, replicated on every partition so any partition
            # base can serve as the matmul contraction row)
            ones2_f = consts.tile([128, 2, 128], F32)
            nc.vector.memset(ones2_f, 0.0)
            nc.vector.memset(ones2_f[:, 0, 0:64], 1.0)
            nc.vector.memset(ones2_f[:, 1, 64:128], 1.0)
            ones2 = consts.tile([128, 2, 128], F32R)
            nc.vector.tensor_copy(ones2, ones2_f)
            onec_f = consts.tile([128, H], F32)
            nc.vector.memset(onec_f, 1.0)
            onec = consts.tile([128, H], BF16)
            nc.vector.tensor_copy(onec, onec_f)
            eps_sb = consts.tile([128, 1], F32)
            nc.vector.memset(eps_sb, 1e-5)
            # pre-place the {Ln, Exp, Copy} table (set 6) so the auto
            # inserter does not thrash between ln-only and exp-only tables
            nc.scalar.add_instruction(mybir.InstLoadActFuncSet(
                name=nc.get_next_instruction_name(), act_func_set_id=6,
                ins=[], outs=[]))

            mv2_sb = resid.tile([128, TOK // 128, 2], F32)
            r1_first = resid.tile([128, 4, C], F32)
            xnT2_first = resid.tile([128, 4, KC, 128], BF16)
            wf1_pre = [resid.tile([128, KC, 512], BF16, name=f"wf1p{i}")
                       for i in range(2)]
            r1_dram = dpool.tile([TOK, C], F32)
            _NC_CACHE["r1_name"] = r1_dram.name

            with (
                tc.tile_pool(name="paw", bufs=1) as paw,
                tc.tile_pool(name="pax", bufs=4) as pax,
                tc.tile_pool(name="pan", bufs=2) as pan,
                tc.tile_pool(name="pat", bufs=4) as pat,
                tc.tile_pool(name="paq", bufs=2) as paq,
                tc.tile_pool(name="pav", bufs=2) as pav,
                tc.tile_pool(name="pae", bufs=2) as pae,
                tc.tile_pool(name="pap", bufs=3) as pap,
                tc.tile_pool(name="pad", bufs=2) as pad_,
                tc.tile_pool(name="par", bufs=2) as par,
                tc.tile_pool(name="pao", bufs=2) as pao,
                tc.tile_pool(name="pas", bufs=2) as pas,
                tc.tile_pool(name="pa1", bufs=2) as pa1,
            ):
                wqkv_sb = paw.tile([128, KC, 3 * C], BF16)
                wproj_sb = paw.tile([128, KC, C], BF16)
                expb_sb_flat = paw.tile([128, NPAIR * 2 * 2 * N], BF16)
                expb_sb = expb_sb_flat.rearrange(
                    "p (g h nk q) -> p g h nk q", g=NPAIR, nk=2, h=2
                )
                cst = (ones2, onec, eps_sb, wqkv_sb, wproj_sb, expb_sb)
                P = (pax, pan, pat, paq, pav, pae, pap, pad_, par, pao,
                     pas, pa1, psW, psS, psO)
                # startup: x(b0), then Q+K weight sections (needed first by
                # batch 0), then LN/transposes, then the remaining residents
                def _mid():
                    nc.sync.dma_start(
                        wqkv_sb[:, :, 0:C],
                        wqkv_d[:, 0:C].rearrange("(k p) n -> p k n", p=128),
                    )
                prep = _prep_a(nc, P, cst, x_d, 0, mid_fn=_mid)
                for sec in (1, 2):
                    nc.sync.dma_start(
                        wqkv_sb[:, :, sec * C:(sec + 1) * C],
                        wqkv_d[:, sec * C:(sec + 1) * C].rearrange(
                            "(k p) n -> p k n", p=128
                        ),
                    )
                nc.sync.dma_start(expb_sb_flat, expb_d[:])
                nc.sync.dma_start(
                    wproj_sb, wproj_d[:].rearrange("(k p) n -> p k n", p=128)
                )
                for b in range(BLOC):
                    nxt_fn = (
                        (lambda bb=b: _prep_a(nc, P, cst, x_d, bb + 1))
                        if b + 1 < BLOC else None
                    )
                    prep = _body_a(nc, P, cst, prep, nxt_fn, r1_dram,
                                   mv2_sb, r1_first, xnT2_first, b)
                    if b == 5:
                        _preload_wf1(nc, wf1_pre, wfc1_d)

            with (
                tc.tile_pool(name="pbr", bufs=9) as pbr,
                tc.tile_pool(name="pbn", bufs=3) as pbn,
                tc.tile_pool(name="pbt", bufs=9) as pbt,
                tc.tile_pool(name="pbw1", bufs=3) as pbw1,
                tc.tile_pool(name="pbw2", bufs=3) as pbw2,
                tc.tile_pool(name="pby", bufs=2) as pby,
            ):
                P = (pbr, pbn, pbt, pbw1, pbw2, pby)
                _phase_b(nc, P, wfc1_d, wfc2_d, r1_dram, mv2_sb, y_d,
                         r1_first, xnT2_first, wf1_pre, psW, psS, psO)

    nc.finalize()
    return nc


def _get_nc():
    if "nc" not in _NC_CACHE:
        _NC_CACHE["nc"] = build()
    return _NC_CACHE["nc"]


def kernel(**inputs):
    x = np.asarray(inputs["x"], dtype=np.float32)
    qkv_w = np.asarray(inputs["qkv_w"], dtype=np.float32)
    qkv_b = np.asarray(inputs["qkv_b"], dtype=np.float32)
    proj_w = np.asarray(inputs["proj_w"], dtype=np.float32)
    proj_b = np.asarray(inputs["proj_b"], dtype=np.float32)
    fc1_w = np.asarray(inputs["fc1_w"], dtype=np.float32)
    fc1_b = np.asarray(inputs["fc1_b"], dtype=np.float32)
    fc2_w = np.asarray(inputs["fc2_w"], dtype=np.float32)
    fc2_b = np.asarray(inputs["fc2_b"], dtype=np.float32)
    ln1_g = np.asarray(inputs["ln1_g"], dtype=np.float32)
    ln1_b = np.asarray(inputs["ln1_b"], dtype=np.float32)
    ln2_g = np.asarray(inputs["ln2_g"], dtype=np.float32)
    ln2_b = np.asarray(inputs["ln2_b"], dtype=np.float32)
    rel_pos_bias = np.asarray(inputs["rel_pos_bias"], dtype=np.float32)
    rel_pos_idx = np.asarray(inputs["rel_pos_idx"])

    assert not np.any(qkv_b) and not np.any(proj_b), "nonzero bias unsupported"
    assert not np.any(fc1_b) and not np.any(fc2_b), "nonzero bias unsupported"
    assert not np.any(ln1_b) and not np.any(ln2_b), "nonzero LN bias unsupported"

    bf = ml_dtypes.bfloat16
    # fold LN gammas into the following weight matrices (exact when g == 1)
    wqkv = np.ascontiguousarray(ln1_g[:, None] * qkv_w).astype(bf)
    wproj = np.ascontiguousarray(proj_w).astype(bf)
    wfc1 = np.ascontiguousarray(ln2_g[:, None] * fc1_w).astype(bf)
    wfc2 = np.ascontiguousarray(fc2_w).astype(bf)

    # expb[p, g, h2, nk, q] = exp(bias[q, nk*128+p, 2g+h2])
    Bm = rel_pos_bias[rel_pos_idx].reshape(N, N, H)          # [q, k, h]
    E = np.exp(Bm).transpose(1, 2, 0)                        # [k, h, q]
    E = E.reshape(2, 128, NPAIR, 2, N).transpose(1, 2, 3, 0, 4)
    expb = np.ascontiguousarray(E.reshape(128, NPAIR * 2 * 2 * N)).astype(bf)

    nc = _get_nc()
    in_maps = []
    for c in range(NCORES):
        xs = np.ascontiguousarray(
            x[c * BLOC:(c + 1) * BLOC].reshape(TOK, C)
        ).astype(np.float32)
        in_maps.append(
            dict(x=xs, wqkv=wqkv, wproj=wproj, wfc1=wfc1, wfc2=wfc2,
                 expb=expb)
        )
    res = run_bass_kernel_spmd(nc, in_maps, core_ids=list(range(NCORES)))
    y = np.concatenate([res.results[c]["y"] for c in range(NCORES)], axis=0)
    return y.reshape(B, N, C).astype(np.float32)
